# revision 37
# baseline (speedup 1.0000x reference)
"""Causal self-attention (B=4, S=4096, D=64, H=4) on 8 TRN2 NeuronCores.

Sharding: the 16 (batch, head) pairs are distributed 2-per-core
(core c -> batch c//2, heads (2*(c%2), 2*(c%2)+1)). Each core runs the
full fused attention for its 2 pairs; no cross-core communication.

Design (evolution of the 215us baseline; see git-less history in
comments):

  - Scores: K=18 matmuls (16 dh dims + 2 exact-bf16 constant
    contraction columns folding t = A*s + B, A = 128/ln2). Chunks of
    CHUNK_BLOCKS key blocks; each block's matmul sits on its own
    32-row PE band (tile_position=(32g,0)) AND its own PSUM bank, so
    the strips stream concurrently (measured: same-band or same-bank
    strips serialize, distinct ones overlap).
  - exp via two engines: ACT-routed spans use Exp with the affine
    folded back out (scale 1/A, bias -B/A); DVE-routed spans use the
    Schraudolph trick: f32->int16 tensor_copy IS the bf16 bit pattern
    of ~exp(s) (verified RNE + saturation).
  - Causal mask: NO mask matmuls. Diagonal-block spans are routed to
    DVE as tensor_mul(ps, tmask) -> int16 bitcast, where tmask[p,c] =
    (c >= p): masked lanes compute int16(t*0) = 0 = bf16 +0.0, an
    exact zero weight. One [128,512] f32 constant serves every
    diagonal block.
  - PV: 4 concurrent N=128 column-band chains per unit accumulating
    into one PSUM bank. No bank-zeroing matmul: the first PV matmul
    of the unit carries start=True, clearing has_written BANK-WIDE
    (probed); the other chains' first writes overwrite-on-cleared.
  - Output: direct DMA from PSUM to DRAM (no SBUF staging copy).
  - Scores stay transposed (key on partitions) so PV needs no
    transpose; softmax denominator = a 17th all-ones column in V; the
    division happens on host. Output [2, 17, 4096] f32 = O.T rows
    0..15 + denominator row 16.
"""

import numpy as np
import ml_dtypes

_B, _S, _D = 4, 4096, 64
_H, _Dh = 4, 16
_NC = 8
_SCALE = 1.0 / np.sqrt(_Dh)
_NQB = _S // 512
_NKB = _S // 128

_A16 = 128.0 / np.log(2.0)
_B16 = 16256.0 - 5.5

# --- config knobs (A/B testing) ---
_CB = 2            # key blocks per scores chunk (PSUM banks per ps tile)
_STRIPS = 1        # score matmuls per block (2 = split q window over 2 bands)
_SPLIT_DIAG = False  # split diag exp into triangle (DVE) + routable remainder
_EXP_SPLIT = False   # split off-diag exp spans across BOTH engines (latency)
_ZERO_MM = False   # emit the K=1 bank-zeroing matmul (else start=True trick)
_DIRECT_OUT = False  # DMA PSUM -> DRAM (unsupported: bass dma_start asserts SBUF/DRAM src)
_FLUSH_EVERY = 8   # PV flush cadence in chunks

_cache = {}


def _build_nc():
    import concourse.tile as tile
    from concourse import bacc, mybir

    bf = mybir.dt.bfloat16
    f32 = mybir.dt.float32
    i16 = mybir.dt.int16
    Exp = mybir.ActivationFunctionType.Exp

    nc = bacc.Bacc("TRN2", target_bir_lowering=False, debug=False, num_devices=_NC)
    xT_d = nc.dram_tensor("xT", [_D + 1, _S], bf, kind="ExternalInput").ap()
    wqk_d = nc.dram_tensor("wqk", [_D + 1, 100], bf, kind="ExternalInput").ap()
    wv_d = nc.dram_tensor("wv", [_D + 1, 32], bf, kind="ExternalInput").ap()
    tm_d = nc.dram_tensor("tm", [128, 512], f32, kind="ExternalInput").ap()
    out_d = nc.dram_tensor("out", [2, 8, 128, 512], f32, kind="ExternalOutput").ap()

    ew = {"act": 2700.0, "dve": 1000.0}

    def route(n_elems):
        a = ew["act"] + (n_elems + 270) / 1.2
        d = ew["dve"] + (n_elems + 80) / 0.90
        if d <= a:
            ew["dve"] = d
            return "dve"
        ew["act"] = a
        return "act"

    QS = [nc.gpsimd, nc.sync, nc.scalar, nc.gpsimd]

    with tile.TileContext(nc) as tc:
        with (
            tc.tile_pool(name="singles", bufs=1) as singles,
            tc.tile_pool(name="ps", bufs=3, space="PSUM") as psp,
            tc.tile_pool(name="stgp", bufs=4) as stgp,
            tc.tile_pool(name="ptp", bufs=12) as ptp,
            tc.tile_pool(name="stg", bufs=4) as stg,
        ):
            xT = singles.tile([_D + 1, _S], bf, tag="xT")
            wqk = singles.tile([_D + 1, 100], bf, tag="wqk")
            wv = singles.tile([_D + 1, 32], bf, tag="wv")
            tmask = singles.tile([128, 512], f32, tag="tm")
            bias_t = singles.tile([128, 1], f32, tag="bias")
            scratch = singles.tile([128, 1], bf, tag="scratch")
            nc.gpsimd.dma_start(out=wqk[:], in_=wqk_d)
            for c in range(4):
                QS[(c + 1) % 3].dma_start(
                    out=xT[:, 1024 * c : 1024 * (c + 1)],
                    in_=xT_d[:, 1024 * c : 1024 * (c + 1)],
                )
            nc.gpsimd.dma_start(out=wv[:], in_=wv_d)
            nc.scalar.dma_start(out=tmask[:], in_=tm_d)
            nc.vector.memset(bias_t[:], float(-_B16 / _A16))
            nc.scalar.activation(out=scratch[:], in_=bias_t[:], func=Exp)

            _SPANS = [(0, 512), (512, 2048), (2048, 4096)]
            qT = [
                [
                    singles.tile([128, s1 - s0], bf, tag=f"qT{p}s{si}", name=f"qT{p}s{si}")
                    for si, (s0, s1) in enumerate(_SPANS)
                ]
                for p in range(2)
            ]
            kT = [
                [
                    singles.tile([128, s1 - s0], bf, tag=f"kT{p}s{si}", name=f"kT{p}s{si}")
                    for si, (s0, s1) in enumerate(_SPANS)
                ]
                for p in range(2)
            ]

            def span_of(c0):
                if c0 < 512:
                    return 0, c0
                if c0 < 2048:
                    return 1, c0 - 512
                return 2, c0 - 2048
            V = [singles.tile([128, _NKB, 17], bf, tag=f"V{p}", name=f"V{p}") for p in range(2)]
            for p in range(2):
                nc.vector.memset(V[p][:], 1.0)

            # ---- QKV projection pieces (emitted interleaved with units) ----
            def proj_half(p, half):
                stage = stgp.tile([50, 2048], bf, tag="st", name="st")
                for cc in range(4):
                    c = 4 * half + cc
                    csl = slice(512 * c, 512 * (c + 1))
                    pq = psp.tile([50, 512], f32, tag="sc", name="pq")
                    nc.tensor.matmul(
                        pq[:],
                        wqk[:, 50 * p : 50 * p + 50],
                        xT[:, csl],
                        start=True,
                        stop=True,
                    )
                    if route(512) == "dve":
                        nc.vector.tensor_copy(
                            stage[:, 512 * cc : 512 * cc + 512], pq[:]
                        )
                    else:
                        nc.scalar.copy(
                            stage[:, 512 * cc : 512 * cc + 512], pq[:]
                        )
                    # fine-grained first slice unblocks qi=0 units early
                    if half == 0 and cc == 0:
                        for g in range(4):
                            QS[g % 3].dma_start(
                                out=qT[p][0][32 * g : 32 * g + 18, 0:512],
                                in_=stage[0:18, 0:512],
                            )
                            QS[(g + 1) % 3].dma_start(
                                out=kT[p][0][32 * g : 32 * g + 18, 0:512],
                                in_=stage[32:50, 0:512],
                            )
                if half == 0:
                    for g in range(4):
                        QS[g % 3].dma_start(
                            out=qT[p][1][32 * g : 32 * g + 18, :],
                            in_=stage[0:18, 512:2048],
                        )
                        QS[(g + 1) % 3].dma_start(
                            out=kT[p][1][32 * g : 32 * g + 18, :],
                            in_=stage[32:50, 512:2048],
                        )
                else:
                    for g in range(4):
                        QS[g % 3].dma_start(
                            out=qT[p][2][32 * g : 32 * g + 18, :], in_=stage[0:18, :]
                        )
                        QS[(g + 1) % 3].dma_start(
                            out=kT[p][2][32 * g : 32 * g + 18, :], in_=stage[32:50, :]
                        )

            def proj_v(batch):
                pv = psp.tile([128, 8, 32], f32, tag="sc", name="pv")
                for t in range(8):
                    s = 8 * batch + t
                    nc.tensor.matmul(
                        pv[:, t, :],
                        xT[:, 128 * s : 128 * (s + 1)],
                        wv[:],
                        start=True,
                        stop=True,
                    )
                nc.vector.tensor_copy(
                    V[0][:, 8 * batch : 8 * batch + 8, 0:16], pv[:, :, 0:16]
                )
                nc.scalar.copy(
                    V[1][:, 8 * batch : 8 * batch + 8, 0:16], pv[:, :, 16:32]
                )
                ew["dve"] += (128 + 80) / 0.90
                ew["act"] += (128 + 270) / 1.2

            # ---- one attention unit: queries [512*qi, 512*(qi+1)) of head p ----
            def unit(p, qi, last=False):
                nkb = 4 * qi + 4
                q0 = 512 * qi
                nchunks = (nkb + _CB - 1) // _CB
                po = psp.tile([128, 512], f32, tag="po", bufs=2, name="po")
                chain_started = [False] * 4
                last_b = [4 * qi + cg for cg in range(4)]
                pv_chunks = []

                def flush_pv(keep):
                    while len(pv_chunks) > keep:
                        for b, pt, pc in pv_chunks.pop(0):
                            j = b - 4 * qi
                            for cg in range(4):
                                if j > cg:
                                    continue
                                nc.tensor.matmul(
                                    po[32 * cg : 32 * cg + 17, 128 * cg : 128 * (cg + 1)],
                                    V[p][:, b, :],
                                    pt[:, pc + 128 * cg : pc + 128 * (cg + 1)],
                                    start=not chain_started[cg],
                                    stop=(b == last_b[cg]),
                                    tile_position=(0, 32 * cg),
                                )
                                chain_started[cg] = True

                border = list(range(nkb))
                for ci in range(nchunks):
                    blks = border[ci * _CB : ci * _CB + _CB]
                    nblk = len(blks)
                    ps = psp.tile([128, 512 * _CB], f32, tag="sc", name="ps")
                    for t, b in enumerate(blks):
                        g = b % 4
                        j = b - 4 * qi
                        off = 128 * j if j > 0 else 0
                        ksi, kc = span_of(128 * b)
                        qsi, qc = span_of(q0 + off)
                        nc.tensor.matmul(
                            ps[:, 512 * t + off : 512 * (t + 1)],
                            kT[p][ksi][32 * g : 32 * g + 18, kc : kc + 128],
                            qT[p][qsi][32 * g : 32 * g + 18, qc : qc + 512 - off],
                            start=True,
                            stop=True,
                            tile_position=(32 * g, 0),
                        )
                    pt = ptp.tile([128, 512 * _CB], bf, tag="pt", name="pt")

                    def exp_dve(lo, hi):
                        ew["dve"] += (hi - lo + 80) / 0.90
                        nc.vector.tensor_copy(
                            pt[:, lo:hi].bitcast(i16), ps[:, lo:hi]
                        )

                    def exp_act(lo, hi):
                        ew["act"] += (hi - lo + 270) / 1.2
                        nc.scalar.activation(
                            out=pt[:, lo:hi],
                            in_=ps[:, lo:hi],
                            func=Exp,
                            bias=bias_t[:],
                            scale=float(1.0 / _A16),
                        )

                    # merge consecutive off-diag blocks into one routable span;
                    # each diagonal block gets a fused mask*exp on DVE
                    run0 = None
                    for t, b in enumerate(list(blks) + [None]):
                        j = (b - 4 * qi) if b is not None else -1
                        if b is not None and j < 0:
                            if run0 is None:
                                run0 = t
                            continue
                        if run0 is not None:
                            lo, hi = 512 * run0, 512 * t
                            if route(hi - lo) == "dve":
                                exp_dve(lo, hi)
                            else:
                                exp_act(lo, hi)
                            run0 = None
                        if b is None:
                            break
                        lo = 512 * t + 128 * j
                        hi = 512 * (t + 1)
                        w = hi - lo
                        # diagonal block: fused causal mask * Schraudolph
                        # exp on DVE (masked lanes -> exact +0.0 weight)
                        ew["dve"] += (w + 80) / 0.90
                        nc.vector.tensor_mul(
                            pt[:, lo:hi].bitcast(i16), ps[:, lo:hi], tmask[:, 0:w]
                        )
                    pv_chunks.append([(b, pt, 512 * t) for t, b in enumerate(blks)])
                    # lagged flush: drain oldest chunks, hold back the freshest
                    # so PV never waits on an in-flight exp
                    if len(pv_chunks) >= _FLUSH_EVERY + 1:
                        flush_pv(1)
                flush_pv(0)
                ost = stg.tile([128, 512], f32, tag="ost", name="ost")
                if route(512) == "dve":
                    nc.vector.tensor_copy(ost[:], po[:])
                else:
                    nc.scalar.copy(ost[:], po[:])
                QS[(2 * qi + p) % 3].dma_start(out=out_d[p][qi], in_=ost[:])

            # ---- emission order: interleave projections with early units so
            # the PE ramps into attention while replication DMAs stream ----
            proj_half(0, 0)
            proj_half(1, 0)
            proj_v(0)
            unit(0, 0)
            unit(1, 0)
            proj_half(0, 1)
            unit(0, 1)
            unit(1, 1)
            proj_v(1)
            unit(0, 2)
            unit(1, 2)
            proj_half(1, 1)
            unit(0, 3)
            unit(1, 3)
            proj_v(2)
            unit(0, 4)
            unit(1, 4)
            proj_v(3)
            for qi in range(5, _NQB):
                unit(0, qi)
                unit(1, qi, last=(qi == _NQB - 1))

    nc.compile()
    return nc


def _get_nc():
    if "nc" not in _cache:
        _cache["nc"] = _build_nc()
    return _cache["nc"]


def _prepare_in_maps(x, Wq, bq, Wk, bk, Wv, bv):
    bf = ml_dtypes.bfloat16
    x = np.asarray(x, np.float32)
    ones = np.ones((1, _S), np.float32)

    def aug(W, b, h, scale=1.0):
        blk = np.concatenate(
            [W[h * _Dh : (h + 1) * _Dh, :], b[h * _Dh : (h + 1) * _Dh, None]], axis=1
        )
        return (blk * scale).T.astype(np.float32)

    tmask = (np.arange(512)[None, :] >= np.arange(128)[:, None]).astype(np.float32)

    in_maps = []
    for c in range(_NC):
        b_idx = c // 2
        heads = (2 * (c % 2), 2 * (c % 2) + 1)
        xT = np.concatenate([x[b_idx].T, ones], axis=0)
        wqk_cols = []
        wv_cols = []
        zeros14 = np.zeros((_D + 1, 14), np.float32)
        for h in heads:
            qe = np.zeros((_D + 1, 18), np.float32)
            qe[:, 0:16] = aug(Wq, bq, h, _SCALE * _A16)
            qe[_D, 16] = 16256.0
            qe[_D, 17] = -5.5
            ke = np.zeros((_D + 1, 18), np.float32)
            ke[:, 0:16] = aug(Wk, bk, h)
            ke[_D, 16] = 1.0
            ke[_D, 17] = 1.0
            wqk_cols.extend([qe, zeros14, ke])
            wv_cols.append(aug(Wv, bv, h))
        in_maps.append(
            {
                "xT": xT.astype(bf),
                "wqk": np.concatenate(wqk_cols, axis=1).astype(bf),
                "wv": np.concatenate(wv_cols, axis=1).astype(bf),
                "tm": tmask,
            }
        )
    return in_maps


def _assemble(results):
    final = np.empty((_B, _S, _D), np.float32)
    ot = np.empty((2, 17, _S), np.float32)
    for c in range(_NC):
        b_idx = c // 2
        for p in range(2):
            h = 2 * (c % 2) + p
            o = np.asarray(results[c]["out"], np.float32)  # [2, 8, 128, 512]
            otv = ot[p].reshape(17, 8, 4, 128)
            for cg in range(4):
                otv[:, :, cg, :] = o[
                    p, :, 32 * cg : 32 * cg + 17, 128 * cg : 128 * (cg + 1)
                ].transpose(1, 0, 2)
            final[b_idx, :, h * _Dh : (h + 1) * _Dh] = (ot[p, :16] / ot[p, 16:17]).T
    return final


def _run(in_maps, trace=False, trace_kwargs=None):
    from concourse.bass_utils import run_bass_kernel_spmd

    nc = _get_nc()
    return run_bass_kernel_spmd(
        nc, in_maps, list(range(_NC)), trace=trace, **(trace_kwargs or {})
    )


def kernel(x, Wq, bq, Wk, bk, Wv, bv):
    in_maps = _prepare_in_maps(x, Wq, bq, Wk, bk, Wv, bv)
    res = _run(in_maps)
    return _assemble(res.results)


# revision 38
# speedup vs baseline: 1.0049x; 1.0049x over previous
"""Causal self-attention (B=4, S=4096, D=64, H=4) on 8 TRN2 NeuronCores.

Sharding: the 16 (batch, head) pairs are distributed 2-per-core
(core c -> batch c//2, heads (2*(c%2), 2*(c%2)+1)). Each core runs the
full fused attention for its 2 pairs; no cross-core communication.

Design (evolution of the 215us baseline; see git-less history in
comments):

  - Scores: K=18 matmuls (16 dh dims + 2 exact-bf16 constant
    contraction columns folding t = A*s + B, A = 128/ln2). Chunks of
    CHUNK_BLOCKS key blocks; each block's matmul sits on its own
    32-row PE band (tile_position=(32g,0)) AND its own PSUM bank, so
    the strips stream concurrently (measured: same-band or same-bank
    strips serialize, distinct ones overlap).
  - exp via two engines: ACT-routed spans use Exp with the affine
    folded back out (scale 1/A, bias -B/A); DVE-routed spans use the
    Schraudolph trick: f32->int16 tensor_copy IS the bf16 bit pattern
    of ~exp(s) (verified RNE + saturation).
  - Causal mask: NO mask matmuls. Diagonal-block spans are routed to
    DVE as tensor_mul(ps, tmask) -> int16 bitcast, where tmask[p,c] =
    (c >= p): masked lanes compute int16(t*0) = 0 = bf16 +0.0, an
    exact zero weight. One [128,512] f32 constant serves every
    diagonal block.
  - PV: 4 concurrent N=128 column-band chains per unit accumulating
    into one PSUM bank. No bank-zeroing matmul: the first PV matmul
    of the unit carries start=True, clearing has_written BANK-WIDE
    (probed); the other chains' first writes overwrite-on-cleared.
  - Output: direct DMA from PSUM to DRAM (no SBUF staging copy).
  - Scores stay transposed (key on partitions) so PV needs no
    transpose; softmax denominator = a 17th all-ones column in V; the
    division happens on host. Output [2, 17, 4096] f32 = O.T rows
    0..15 + denominator row 16.
"""

import numpy as np
import ml_dtypes

_B, _S, _D = 4, 4096, 64
_H, _Dh = 4, 16
_NC = 8
_SCALE = 1.0 / np.sqrt(_Dh)
_NQB = _S // 512
_NKB = _S // 128

_A16 = 128.0 / np.log(2.0)
_B16 = 16256.0 - 5.5

# --- config knobs (A/B testing) ---
_CB = 2            # key blocks per scores chunk (PSUM banks per ps tile)
_STRIPS = 1        # score matmuls per block (2 = split q window over 2 bands)
_SPLIT_DIAG = False  # split diag exp into triangle (DVE) + routable remainder
_EXP_SPLIT = False   # split off-diag exp spans across BOTH engines (latency)
_ZERO_MM = False   # emit the K=1 bank-zeroing matmul (else start=True trick)
_DIRECT_OUT = False  # DMA PSUM -> DRAM (unsupported: bass dma_start asserts SBUF/DRAM src)
_FLUSH_EVERY = 8   # PV flush cadence in chunks

_cache = {}


def _build_nc():
    import concourse.tile as tile
    from concourse import bacc, mybir

    bf = mybir.dt.bfloat16
    f32 = mybir.dt.float32
    i16 = mybir.dt.int16
    Exp = mybir.ActivationFunctionType.Exp

    nc = bacc.Bacc("TRN2", target_bir_lowering=False, debug=False, num_devices=_NC)
    xT_d = nc.dram_tensor("xT", [_D + 1, _S], bf, kind="ExternalInput").ap()
    wqk_d = nc.dram_tensor("wqk", [_D + 1, 100], bf, kind="ExternalInput").ap()
    wv_d = nc.dram_tensor("wv", [_D + 1, 32], bf, kind="ExternalInput").ap()
    tm_d = nc.dram_tensor("tm", [128, 512], f32, kind="ExternalInput").ap()
    out_d = nc.dram_tensor("out", [2, 8, 128, 512], f32, kind="ExternalOutput").ap()

    ew = {"act": 2700.0, "dve": 1000.0}

    def route(n_elems):
        a = ew["act"] + (n_elems + 270) / 1.2
        d = ew["dve"] + (n_elems + 80) / 0.90
        if d <= a:
            ew["dve"] = d
            return "dve"
        ew["act"] = a
        return "act"

    QS = [nc.gpsimd, nc.sync, nc.scalar, nc.gpsimd]

    with tile.TileContext(nc) as tc:
        with (
            tc.tile_pool(name="singles", bufs=1) as singles,
            tc.tile_pool(name="ps", bufs=3, space="PSUM") as psp,
            tc.tile_pool(name="stgp", bufs=4) as stgp,
            tc.tile_pool(name="ptp", bufs=10) as ptp,
            tc.tile_pool(name="stg", bufs=3) as stg,
        ):
            xT = singles.tile([_D + 1, _S], bf, tag="xT")
            wqk = singles.tile([_D + 1, 100], bf, tag="wqk")
            wv = singles.tile([_D + 1, 32], bf, tag="wv")
            tmask = singles.tile([128, 512], f32, tag="tm")
            bias_t = singles.tile([128, 1], f32, tag="bias")
            scratch = singles.tile([128, 1], bf, tag="scratch")
            nc.gpsimd.dma_start(out=wqk[:], in_=wqk_d)
            for c in range(4):
                QS[(c + 1) % 3].dma_start(
                    out=xT[:, 1024 * c : 1024 * (c + 1)],
                    in_=xT_d[:, 1024 * c : 1024 * (c + 1)],
                )
            nc.gpsimd.dma_start(out=wv[:], in_=wv_d)
            nc.scalar.dma_start(out=tmask[:], in_=tm_d)
            nc.vector.memset(bias_t[:], float(-_B16 / _A16))
            nc.scalar.activation(out=scratch[:], in_=bias_t[:], func=Exp)

            _SPANS = [(0, 512), (512, 2048), (2048, 4096)]
            qT = [
                [
                    singles.tile([128, s1 - s0], bf, tag=f"qT{p}s{si}", name=f"qT{p}s{si}")
                    for si, (s0, s1) in enumerate(_SPANS)
                ]
                for p in range(2)
            ]
            kT = [
                [
                    singles.tile([128, s1 - s0], bf, tag=f"kT{p}s{si}", name=f"kT{p}s{si}")
                    for si, (s0, s1) in enumerate(_SPANS)
                ]
                for p in range(2)
            ]

            def span_of(c0):
                if c0 < 512:
                    return 0, c0
                if c0 < 2048:
                    return 1, c0 - 512
                return 2, c0 - 2048
            V = [singles.tile([128, _NKB, 17], bf, tag=f"V{p}", name=f"V{p}") for p in range(2)]
            for p in range(2):
                nc.vector.memset(V[p][:], 1.0)

            # ---- QKV projection pieces (emitted interleaved with units) ----
            def proj_half(p, half):
                stage = stgp.tile([50, 2048], bf, tag="st", name="st")
                for cc in range(4):
                    c = 4 * half + cc
                    csl = slice(512 * c, 512 * (c + 1))
                    pq = psp.tile([50, 512], f32, tag="sc", name="pq")
                    nc.tensor.matmul(
                        pq[:],
                        wqk[:, 50 * p : 50 * p + 50],
                        xT[:, csl],
                        start=True,
                        stop=True,
                    )
                    if route(512) == "dve":
                        nc.vector.tensor_copy(
                            stage[:, 512 * cc : 512 * cc + 512], pq[:]
                        )
                    else:
                        nc.scalar.copy(
                            stage[:, 512 * cc : 512 * cc + 512], pq[:]
                        )
                    # fine-grained first slice unblocks qi=0 units early
                    if half == 0 and cc == 0:
                        for g in range(4):
                            QS[g % 3].dma_start(
                                out=qT[p][0][32 * g : 32 * g + 18, 0:512],
                                in_=stage[0:18, 0:512],
                            )
                            QS[(g + 1) % 3].dma_start(
                                out=kT[p][0][32 * g : 32 * g + 18, 0:512],
                                in_=stage[32:50, 0:512],
                            )
                if half == 0:
                    for g in range(4):
                        QS[g % 3].dma_start(
                            out=qT[p][1][32 * g : 32 * g + 18, :],
                            in_=stage[0:18, 512:2048],
                        )
                        QS[(g + 1) % 3].dma_start(
                            out=kT[p][1][32 * g : 32 * g + 18, :],
                            in_=stage[32:50, 512:2048],
                        )
                else:
                    for g in range(4):
                        QS[g % 3].dma_start(
                            out=qT[p][2][32 * g : 32 * g + 18, :], in_=stage[0:18, :]
                        )
                        QS[(g + 1) % 3].dma_start(
                            out=kT[p][2][32 * g : 32 * g + 18, :], in_=stage[32:50, :]
                        )

            def proj_v(batch):
                pv = psp.tile([128, 8, 32], f32, tag="sc", name="pv")
                for t in range(8):
                    s = 8 * batch + t
                    nc.tensor.matmul(
                        pv[:, t, :],
                        xT[:, 128 * s : 128 * (s + 1)],
                        wv[:],
                        start=True,
                        stop=True,
                    )
                nc.vector.tensor_copy(
                    V[0][:, 8 * batch : 8 * batch + 8, 0:16], pv[:, :, 0:16]
                )
                nc.scalar.copy(
                    V[1][:, 8 * batch : 8 * batch + 8, 0:16], pv[:, :, 16:32]
                )
                ew["dve"] += (128 + 80) / 0.90
                ew["act"] += (128 + 270) / 1.2

            # ---- one attention unit: queries [512*qi, 512*(qi+1)) of head p ----
            def unit(p, qi, last=False):
                nkb = 4 * qi + 4
                q0 = 512 * qi
                nchunks = (nkb + _CB - 1) // _CB
                po = psp.tile([128, 512], f32, tag="po", bufs=2, name="po")
                chain_started = [False] * 4
                last_b = [4 * qi + cg for cg in range(4)]
                pv_chunks = []

                def flush_pv(keep):
                    while len(pv_chunks) > keep:
                        for b, pt, pc in pv_chunks.pop(0):
                            j = b - 4 * qi
                            for cg in range(4):
                                if j > cg:
                                    continue
                                nc.tensor.matmul(
                                    po[32 * cg : 32 * cg + 17, 128 * cg : 128 * (cg + 1)],
                                    V[p][:, b, :],
                                    pt[:, pc + 128 * cg : pc + 128 * (cg + 1)],
                                    start=not chain_started[cg],
                                    stop=(b == last_b[cg]),
                                    tile_position=(0, 32 * cg),
                                )
                                chain_started[cg] = True

                border = list(range(nkb))
                for ci in range(nchunks):
                    blks = border[ci * _CB : ci * _CB + _CB]
                    nblk = len(blks)
                    ps = psp.tile([128, 512 * _CB], f32, tag="sc", name="ps")
                    for t, b in enumerate(blks):
                        g = b % 4
                        j = b - 4 * qi
                        off = 128 * j if j > 0 else 0
                        ksi, kc = span_of(128 * b)
                        qsi, qc = span_of(q0 + off)
                        nc.tensor.matmul(
                            ps[:, 512 * t + off : 512 * (t + 1)],
                            kT[p][ksi][32 * g : 32 * g + 18, kc : kc + 128],
                            qT[p][qsi][32 * g : 32 * g + 18, qc : qc + 512 - off],
                            start=True,
                            stop=True,
                            tile_position=(32 * g, 0),
                        )
                    pt = ptp.tile([128, 512 * _CB], bf, tag="pt", name="pt")

                    def exp_dve(lo, hi):
                        ew["dve"] += (hi - lo + 80) / 0.90
                        nc.vector.tensor_copy(
                            pt[:, lo:hi].bitcast(i16), ps[:, lo:hi]
                        )

                    def exp_act(lo, hi):
                        ew["act"] += (hi - lo + 270) / 1.2
                        nc.scalar.activation(
                            out=pt[:, lo:hi],
                            in_=ps[:, lo:hi],
                            func=Exp,
                            bias=bias_t[:],
                            scale=float(1.0 / _A16),
                        )

                    # merge consecutive off-diag blocks into one routable span;
                    # each diagonal block gets a fused mask*exp on DVE
                    run0 = None
                    for t, b in enumerate(list(blks) + [None]):
                        j = (b - 4 * qi) if b is not None else -1
                        if b is not None and j < 0:
                            if run0 is None:
                                run0 = t
                            continue
                        if run0 is not None:
                            lo, hi = 512 * run0, 512 * t
                            if route(hi - lo) == "dve":
                                exp_dve(lo, hi)
                            else:
                                exp_act(lo, hi)
                            run0 = None
                        if b is None:
                            break
                        lo = 512 * t + 128 * j
                        hi = 512 * (t + 1)
                        w = hi - lo
                        # diagonal block: fused causal mask * Schraudolph
                        # exp on DVE (masked lanes -> exact +0.0 weight)
                        ew["dve"] += (w + 80) / 0.90
                        nc.vector.tensor_mul(
                            pt[:, lo:hi].bitcast(i16), ps[:, lo:hi], tmask[:, 0:w]
                        )
                    pv_chunks.append([(b, pt, 512 * t) for t, b in enumerate(blks)])
                    # lagged flush: drain oldest chunks, hold back the freshest
                    # so PV never waits on an in-flight exp
                    if len(pv_chunks) >= _FLUSH_EVERY + 1:
                        flush_pv(1)
                flush_pv(0)
                ost = stg.tile([128, 512], f32, tag="ost", name="ost")
                if route(512) == "dve":
                    nc.vector.tensor_copy(ost[:], po[:])
                else:
                    nc.scalar.copy(ost[:], po[:])
                QS[(2 * qi + p) % 3].dma_start(out=out_d[p][qi], in_=ost[:])

            # ---- emission order: interleave projections with early units so
            # the PE ramps into attention while replication DMAs stream ----
            proj_half(0, 0)
            proj_half(1, 0)
            proj_v(0)
            unit(0, 0)
            unit(1, 0)
            proj_half(0, 1)
            unit(0, 1)
            unit(1, 1)
            proj_v(1)
            unit(0, 2)
            unit(1, 2)
            proj_half(1, 1)
            unit(0, 3)
            unit(1, 3)
            proj_v(2)
            unit(0, 4)
            unit(1, 4)
            proj_v(3)
            for qi in range(5, _NQB):
                unit(0, qi)
                unit(1, qi, last=(qi == _NQB - 1))

    nc.compile()
    return nc


def _get_nc():
    if "nc" not in _cache:
        _cache["nc"] = _build_nc()
    return _cache["nc"]


def _prepare_in_maps(x, Wq, bq, Wk, bk, Wv, bv):
    bf = ml_dtypes.bfloat16
    x = np.asarray(x, np.float32)
    ones = np.ones((1, _S), np.float32)

    def aug(W, b, h, scale=1.0):
        blk = np.concatenate(
            [W[h * _Dh : (h + 1) * _Dh, :], b[h * _Dh : (h + 1) * _Dh, None]], axis=1
        )
        return (blk * scale).T.astype(np.float32)

    tmask = (np.arange(512)[None, :] >= np.arange(128)[:, None]).astype(np.float32)

    in_maps = []
    for c in range(_NC):
        b_idx = c // 2
        heads = (2 * (c % 2), 2 * (c % 2) + 1)
        xT = np.concatenate([x[b_idx].T, ones], axis=0)
        wqk_cols = []
        wv_cols = []
        zeros14 = np.zeros((_D + 1, 14), np.float32)
        for h in heads:
            qe = np.zeros((_D + 1, 18), np.float32)
            qe[:, 0:16] = aug(Wq, bq, h, _SCALE * _A16)
            qe[_D, 16] = 16256.0
            qe[_D, 17] = -5.5
            ke = np.zeros((_D + 1, 18), np.float32)
            ke[:, 0:16] = aug(Wk, bk, h)
            ke[_D, 16] = 1.0
            ke[_D, 17] = 1.0
            wqk_cols.extend([qe, zeros14, ke])
            wv_cols.append(aug(Wv, bv, h))
        in_maps.append(
            {
                "xT": xT.astype(bf),
                "wqk": np.concatenate(wqk_cols, axis=1).astype(bf),
                "wv": np.concatenate(wv_cols, axis=1).astype(bf),
                "tm": tmask,
            }
        )
    return in_maps


def _assemble(results):
    final = np.empty((_B, _S, _D), np.float32)
    ot = np.empty((2, 17, _S), np.float32)
    for c in range(_NC):
        b_idx = c // 2
        for p in range(2):
            h = 2 * (c % 2) + p
            o = np.asarray(results[c]["out"], np.float32)  # [2, 8, 128, 512]
            otv = ot[p].reshape(17, 8, 4, 128)
            for cg in range(4):
                otv[:, :, cg, :] = o[
                    p, :, 32 * cg : 32 * cg + 17, 128 * cg : 128 * (cg + 1)
                ].transpose(1, 0, 2)
            final[b_idx, :, h * _Dh : (h + 1) * _Dh] = (ot[p, :16] / ot[p, 16:17]).T
    return final


def _run(in_maps, trace=False, trace_kwargs=None):
    from concourse.bass_utils import run_bass_kernel_spmd

    nc = _get_nc()
    return run_bass_kernel_spmd(
        nc, in_maps, list(range(_NC)), trace=trace, **(trace_kwargs or {})
    )


def kernel(x, Wq, bq, Wk, bk, Wv, bv):
    in_maps = _prepare_in_maps(x, Wq, bq, Wk, bk, Wv, bv)
    res = _run(in_maps)
    return _assemble(res.results)
